# revision 12
# baseline (speedup 1.0000x reference)
"""GATv2 block kernel for 8 Trainium2 NeuronCores (Bass/Tile) — v2.

Strategy (graph/data parallel over destination nodes):
  - Host sorts edges by destination, shards destination nodes across the
    8 cores (6250 nodes each, padded to 6272 = 49 tiles of 128).
  - Per destination-node tile, edges are padded to multiples of 128
    ("chunks"); chunk counts per tile are maxed across cores so one SPMD
    program serves all 8 cores.
  - Host supplies x[src] AND x[dst] pre-gathered + transposed (fp8) so
    every linear transform on device is a matmul with a constant
    stationary operand (no indicator gathers, no partition broadcasts).
  - fp8 stationaries get 4x fast-weight-load; segment softmax + scatter
    are matmuls against an edge->node indicator built by GpSimd.
"""

import numpy as np
import ml_dtypes

BF16 = ml_dtypes.bfloat16
FP8 = ml_dtypes.float8_e4m3

P = 128
HEADS = 4
HEAD_DIM = 32
OUT_DIM = 128
IN_DIM = 128
EDGE_DIM = 10
NEG_SLOPE = 0.2
LN_EPS = 1e-5
N_CORES = 8
SUPER = 4  # chunks per superchunk (free dim 512)

# engine for the indicator build: 'gpsimd' or 'vector'
I_ENGINE = 'vector'

_CACHE = {}

_PATCHED = []


def _enable_ldw_opt():
    # walrus LDWEIGHTS double-buffering: lets weight loads overlap in-flight
    # matmuls instead of serializing every LDW+MM pair.
    if _PATCHED:
        return
    from concourse import bass_utils as bu
    orig = bu.run_command

    def run_command(argv, **kwargs):
        argv = ['--enable-ldw-opt=true' if a == '--enable-ldw-opt=false' else a
                for a in argv]
        return orig(argv, **kwargs)

    bu.run_command = run_command
    _PATCHED.append(True)


def _build_program(C_list, trivial_affine):
    import concourse.bacc as bacc
    import concourse.bass as bass
    import concourse.tile as tile
    from concourse import mybir

    f32 = mybir.dt.float32
    bf16 = mybir.dt.bfloat16
    fp8 = mybir.dt.float8e4
    AT = mybir.ActivationFunctionType
    OP = mybir.AluOpType

    NT = len(C_list)                       # node tiles per core
    CMAX = max(C_list)
    TOTAL_CHUNKS = sum(C_list)
    NPC_PAD = NT * P
    EW = TOTAL_CHUNKS * P                  # padded edges per core

    nc = bacc.Bacc('TRN2', target_bir_lowering=False, debug=False,
                   enable_asserts=True, num_devices=N_CORES)

    # ---- external inputs ----
    x_srcT = nc.dram_tensor('x_srcT', [P, EW], fp8, kind='ExternalInput')
    x_dstT = nc.dram_tensor('x_dstT', [P, EW], fp8, kind='ExternalInput')
    attrT = nc.dram_tensor('attrT', [EDGE_DIM, EW], fp8, kind='ExternalInput')
    ind = nc.dram_tensor('ind', [P, EW], fp8, kind='ExternalInput')
    x_own = nc.dram_tensor('x_own', [NPC_PAD, P], f32, kind='ExternalInput')
    w_lT = nc.dram_tensor('w_lT', [P, P], fp8, kind='ExternalInput')
    w_rT = nc.dram_tensor('w_rT', [P, P], fp8, kind='ExternalInput')
    w_eT = nc.dram_tensor('w_eT', [EDGE_DIM, P], fp8, kind='ExternalInput')
    att_exp = nc.dram_tensor('att_exp', [P, HEADS], fp8, kind='ExternalInput')
    bias_lr = nc.dram_tensor('bias_lr', [P, 1], f32, kind='ExternalInput')
    aff = None
    if not trivial_affine:
        # rows: b_l bcast, conv_bias bcast, gamma bcast, beta bcast
        aff = nc.dram_tensor('aff', [P, 4 * P], f32, kind='ExternalInput')

    out_d = nc.dram_tensor('out', [NPC_PAD, P], f32, kind='ExternalOutput')

    from concourse import library_config
    with tile.TileContext(nc) as tc:
        nc.gpsimd.load_library(library_config.standard)
        with tc.tile_pool(name='const', bufs=1) as cp:
            c_wlT = cp.tile([P, P], fp8)
            nc.sync.dma_start(c_wlT[:], w_lT[:])
            c_wrT = cp.tile([P, P], fp8)
            nc.sync.dma_start(c_wrT[:], w_rT[:])
            c_weT = cp.tile([EDGE_DIM, P], fp8)
            nc.sync.dma_start(c_weT[:], w_eT[:])
            c_att = cp.tile([P, HEADS], fp8)
            nc.sync.dma_start(c_att[:], att_exp[:])
            c_blr = cp.tile([P, 1], f32)
            nc.sync.dma_start(c_blr[:], bias_lr[:])
            c_aff = None
            if aff is not None:
                c_aff = cp.tile([P, 4 * P], f32)
                nc.sync.dma_start(c_aff[:], aff[:])

            with tc.tile_pool(name='persist', bufs=1) as pp:
                ubuf = pp.tile([P, NT * 132], f32)     # unnorm(128)+denom(4)
                hbuf = pp.tile([P, NT * P], f32)       # post-residual h
                stats = pp.tile([P, NT * 2], f32)      # mean, var interleaved

                # ---------- edge pipeline ----------
                with tc.tile_pool(name='eload', bufs=4) as lp, \
                     tc.tile_pool(name='ework', bufs=4) as wp, \
                     tc.tile_pool(name='psA', bufs=2, space='PSUM') as psA, \
                     tc.tile_pool(name='psX', bufs=2, space='PSUM') as psX, \
                     tc.tile_pool(name='psC', bufs=2, space='PSUM') as psC, \
                     tc.tile_pool(name='psO', bufs=2, space='PSUM') as psO:
                    chunk_base = 0
                    for t in range(NT):
                        Ct = C_list[t]
                        te0 = chunk_base * P
                        TW = Ct * P
                        xsT_t = lp.tile([P, CMAX * P], fp8, tag='xsT')
                        nc.sync.dma_start(xsT_t[:, :TW], x_srcT[:, te0:te0 + TW])
                        xdT_t = lp.tile([P, CMAX * P], fp8, tag='xdT')
                        nc.sync.dma_start(xdT_t[:, :TW], x_dstT[:, te0:te0 + TW])
                        atr_t = lp.tile([EDGE_DIM, CMAX * P], fp8, tag='atr')
                        nc.sync.dma_start(atr_t[:, :TW], attrT[:, te0:te0 + TW])
                        # indicator I[e, c, n] = (dst_local[e,c] == n),
                        # prebuilt on host
                        I_t = lp.tile([P, CMAX, P], fp8, tag='I')
                        nc.sync.dma_start(
                            I_t[:, :Ct, :].rearrange('p c n -> p (c n)'),
                            ind[:, te0:te0 + TW])

                        ps_out = psO.tile([P, 132], f32, tag='out')
                        n_super = (Ct + SUPER - 1) // SUPER
                        for s in range(n_super):
                            nch = min(SUPER, Ct - s * SUPER)
                            W = nch * P
                            o0 = s * SUPER * P
                            xsT = xsT_t[:, o0:o0 + W]
                            xdT = xdT_t[:, o0:o0 + W]
                            atr = atr_t[:, o0:o0 + W]

                            # s^T = xl[src]^T + ea^T + xr[dst]^T (feature-major)
                            ps_sT = psA.tile([P, SUPER * P], f32, tag='sT')
                            nc.tensor.matmul(ps_sT[:, :W], lhsT=c_wlT[:],
                                             rhs=xsT, start=True, stop=False)
                            nc.tensor.matmul(ps_sT[:, :W], lhsT=c_weT[:],
                                             rhs=atr, start=False, stop=False)
                            nc.tensor.matmul(ps_sT[:, :W], lhsT=c_wrT[:],
                                             rhs=xdT, start=False, stop=True)

                            # m = lrelu(s + (b_l+b_r))  (bias per feature row)
                            m = wp.tile([P, SUPER * P], fp8, tag='m')
                            nc.scalar.activation(m[:, :W], ps_sT[:, :W], AT.Prelu,
                                                 bias=c_blr[:], alpha=NEG_SLOPE)

                            # logits edge-major: [128e, 4] per chunk
                            ps_ex = psC.tile([P, SUPER * HEADS], f32, tag='lgex')
                            for j in range(nch):
                                nc.tensor.matmul(
                                    ps_ex[:, j * HEADS:(j + 1) * HEADS],
                                    lhsT=m[:, j * P:(j + 1) * P],
                                    rhs=c_att[:], start=True, stop=True)

                            # msg = [xj * ex_bcast | ex]  -> [128, nch, 132]
                            msg = wp.tile([P, SUPER, 132], fp8, tag='msg')
                            # ex = exp(logits), straight into msg cols 128:132
                            nc.scalar.activation(
                                msg[:, :nch, P:P + HEADS],
                                ps_ex[:, :nch * HEADS].rearrange(
                                    'p (c h) -> p c h', c=nch),
                                AT.Exp)

                            # xj edge-major [e, f]
                            ps_xj = psX.tile([P, SUPER * P], f32, tag='xj')
                            for j in range(nch):
                                nc.tensor.matmul(
                                    ps_xj[:, j * P:(j + 1) * P],
                                    lhsT=xsT[:, j * P:(j + 1) * P],
                                    rhs=c_wlT[:], start=True, stop=True)

                            xj_v = ps_xj[:, :W].rearrange(
                                'p (c h d) -> p c h d', c=nch, h=HEADS)
                            if aff is not None:
                                # general b_l: xj += b_l (broadcast over rows)
                                xj_sb = wp.tile([P, SUPER * P], f32, tag='xjb')
                                blv = c_aff[:, 0:P][:, None, :].to_broadcast(
                                    [P, nch, P])
                                nc.vector.tensor_tensor(
                                    out=xj_sb[:, :W].rearrange(
                                        'p (c f) -> p c f', c=nch),
                                    in0=ps_xj[:, :W].rearrange(
                                        'p (c f) -> p c f', c=nch),
                                    in1=blv, op=OP.add)
                                xj_v = xj_sb[:, :W].rearrange(
                                    'p (c h d) -> p c h d', c=nch, h=HEADS)
                            ex_v = (msg[:, :nch, P:P + HEADS]
                                    [:, :, :, None].to_broadcast(
                                        [P, nch, HEADS, HEAD_DIM]))
                            nc.vector.tensor_tensor(
                                out=msg[:, :nch, 0:P].rearrange(
                                    'p c (h d) -> p c h d', h=HEADS),
                                in0=xj_v, in1=ex_v, op=OP.mult)

                            # scatter: ps_out[n, :] += I^T @ msg
                            for j in range(nch):
                                first = (s == 0 and j == 0)
                                last = (s == n_super - 1 and j == nch - 1)
                                nc.tensor.matmul(ps_out[:],
                                                 lhsT=I_t[:, s * SUPER + j, :],
                                                 rhs=msg[:, j, :],
                                                 start=first, stop=last)
                        nc.scalar.copy(
                            ubuf[:, t * 132:(t + 1) * 132], ps_out[:])
                        chunk_base += Ct

                # ---------- tail: normalize + silu + residual + LN ----------
                with tc.tile_pool(name='tail', bufs=3) as tp:
                    for t in range(NT):
                        u_sl = ubuf[:, t * 132:t * 132 + P]
                        d_sl = ubuf[:, t * 132 + P:t * 132 + P + HEADS]
                        rv = tp.tile([P, HEADS], f32, tag='rv')
                        nc.vector.tensor_scalar(
                            out=rv[:], in0=d_sl, scalar1=1e-16, scalar2=None,
                            op0=OP.add)
                        rvi = tp.tile([P, HEADS], f32, tag='rvi')
                        nc.vector.reciprocal(rvi[:], rv[:])
                        u = tp.tile([P, P], f32, tag='u')
                        rvi_v = rvi[:, :, None].to_broadcast(
                            [P, HEADS, HEAD_DIM])
                        nc.vector.tensor_tensor(
                            out=u[:].rearrange('p (h d) -> p h d', h=HEADS),
                            in0=u_sl.rearrange('p (h d) -> p h d', h=HEADS),
                            in1=rvi_v, op=OP.mult)
                        if aff is not None:
                            nc.vector.tensor_tensor(
                                out=u[:], in0=u[:], in1=c_aff[:, P:2 * P],
                                op=OP.add)
                        ss = tp.tile([P, P], f32, tag='ss')
                        nc.scalar.activation(ss[:], u[:], AT.Silu)
                        xo = tp.tile([P, P], f32, tag='xo')
                        nc.gpsimd.dma_start(xo[:], x_own[t * P:(t + 1) * P, :])
                        h_sl = hbuf[:, t * P:(t + 1) * P]
                        nc.vector.tensor_tensor(out=h_sl, in0=ss[:], in1=xo[:],
                                                op=OP.add)
                        bs = tp.tile([P, 6], f32, tag='bs')
                        nc.vector.bn_stats(bs[:], h_sl)
                        nc.vector.bn_aggr(stats[:, t * 2:t * 2 + 2], bs[:])

                    veps = tp.tile([P, NT], f32, tag='veps')
                    var_v = stats[:].rearrange('p (t k) -> p t k', k=2)[:, :, 1]
                    nc.vector.tensor_scalar(out=veps[:], in0=var_v,
                                            scalar1=LN_EPS, scalar2=None,
                                            op0=OP.add)
                    vinv = tp.tile([P, NT], f32, tag='vinv')
                    nc.vector.reciprocal(vinv[:], veps[:])
                    rstd = tp.tile([P, NT], f32, tag='rstd')
                    nc.scalar.activation(rstd[:], vinv[:], AT.Sqrt)

                    for t in range(NT):
                        o = tp.tile([P, P], f32, tag='o')
                        nc.vector.tensor_scalar(
                            out=o[:], in0=hbuf[:, t * P:(t + 1) * P],
                            scalar1=stats[:, t * 2:t * 2 + 1],
                            scalar2=rstd[:, t:t + 1],
                            op0=OP.subtract, op1=OP.mult)
                        if aff is not None:
                            nc.vector.tensor_tensor(
                                out=o[:], in0=o[:], in1=c_aff[:, 2 * P:3 * P],
                                op=OP.mult)
                            nc.vector.tensor_tensor(
                                out=o[:], in0=o[:], in1=c_aff[:, 3 * P:4 * P],
                                op=OP.add)
                        nc.gpsimd.dma_start(out_d[t * P:(t + 1) * P, :], o[:])

    nc.compile()
    return nc


def kernel(x, edge_index, edge_attr, w_l, b_l, w_r, b_r, w_e, att,
           conv_bias, ln_gamma, ln_beta):
    from concourse.bass_utils import run_bass_kernel_spmd

    x = np.asarray(x, dtype=np.float32)
    edge_index = np.asarray(edge_index)
    edge_attr = np.asarray(edge_attr, dtype=np.float32)
    w_l = np.asarray(w_l, dtype=np.float32)
    b_l = np.asarray(b_l, dtype=np.float32)
    w_r = np.asarray(w_r, dtype=np.float32)
    b_r = np.asarray(b_r, dtype=np.float32)
    w_e = np.asarray(w_e, dtype=np.float32)
    att = np.asarray(att, dtype=np.float32)
    conv_bias = np.asarray(conv_bias, dtype=np.float32)
    ln_gamma = np.asarray(ln_gamma, dtype=np.float32)
    ln_beta = np.asarray(ln_beta, dtype=np.float32)

    N = x.shape[0]
    E = edge_index.shape[1]
    NPC = (N + N_CORES - 1) // N_CORES          # 6250
    NT = (NPC + P - 1) // P                     # 49
    NPC_PAD = NT * P                            # 6272

    src = edge_index[0].astype(np.int64)
    dst = edge_index[1].astype(np.int64)
    core = np.minimum(dst // NPC, N_CORES - 1)

    trivial_affine = (not b_l.any()) and (not conv_bias.any()) and \
        np.all(ln_gamma == 1.0) and (not ln_beta.any())

    # per (core, tile) edge lists, sorted by dst
    order = np.lexsort((dst,))
    src_s, dst_s, core_s = src[order], dst[order], core[order]
    attr_s = edge_attr[order]
    tile_of = (dst_s - core_s * NPC) // P

    counts = np.zeros((N_CORES, NT), dtype=np.int64)
    np.add.at(counts, (core_s, tile_of), 1)
    C_list = [int(max(1, np.max((counts[:, t] + P - 1) // P)))
              for t in range(NT)]
    TOTAL_CHUNKS = sum(C_list)
    EW = TOTAL_CHUNKS * P

    key = (tuple(C_list), trivial_affine)
    if key in _CACHE:
        nc = _CACHE[key]
    else:
        nc = _build_program(C_list, trivial_affine)
        _CACHE[key] = nc

    # chunk start offsets per tile
    tile_chunk0 = np.zeros(NT, dtype=np.int64)
    acc = 0
    for t in range(NT):
        tile_chunk0[t] = acc
        acc += C_list[t]

    # consts shared by all cores
    w_lT_h = np.ascontiguousarray(w_l.T).astype(FP8)
    w_rT_h = np.ascontiguousarray(w_r.T).astype(FP8)
    w_eT_h = np.ascontiguousarray(w_e.T).astype(FP8)
    att_exp_h = np.zeros((P, HEADS), dtype=FP8)
    for h in range(HEADS):
        att_exp_h[h * HEAD_DIM:(h + 1) * HEAD_DIM, h] = att[h]
    bias_lr_h = (b_l + b_r)[:, None].astype(np.float32).copy()
    aff_h = None
    if not trivial_affine:
        aff_h = np.concatenate([
            np.broadcast_to(b_l, (P, P)),
            np.broadcast_to(conv_bias, (P, P)),
            np.broadcast_to(ln_gamma, (P, P)),
            np.broadcast_to(ln_beta, (P, P))], axis=1).astype(np.float32).copy()

    x_fp8 = x.astype(FP8)

    in_maps = []
    for k in range(N_CORES):
        sel = core_s == k
        ksrc, kdst, ktile = src_s[sel], dst_s[sel], tile_of[sel]
        kattr = attr_s[sel]
        # position of each edge in the padded layout
        # edges already sorted by dst -> grouped by tile, in order
        pos = np.empty(len(ksrc), dtype=np.int64)
        x_srcT_h = np.zeros((P, EW), dtype=FP8)
        x_dstT_h = np.zeros((P, EW), dtype=FP8)
        attrT_h = np.zeros((EDGE_DIM, EW), dtype=FP8)
        ind_h = np.zeros((P, EW), dtype=FP8)
        for t in range(NT):
            tsel = ktile == t
            n_t = int(tsel.sum())
            base = tile_chunk0[t] * P
            pos[tsel] = base + np.arange(n_t)
        x_srcT_h[:, pos] = x_fp8[ksrc].T
        x_dstT_h[:, pos] = x_fp8[kdst].T
        attrT_h[:, pos] = kattr.T.astype(FP8)
        dloc = (kdst - k * NPC - ktile * P).astype(np.int64)
        # indicator: edge at padded slot pos (row pos%128, chunk pos//128)
        # points at local node dloc -> column (pos//128)*128 + dloc
        ind_h[pos % P, (pos // P) * P + dloc] = 1.0

        xk = np.zeros((NPC_PAD, P), dtype=np.float32)
        n_own = min(NPC, N - k * NPC)
        xk[:n_own] = x[k * NPC:k * NPC + n_own]
        im = {
            'x_srcT': x_srcT_h, 'x_dstT': x_dstT_h, 'attrT': attrT_h,
            'ind': ind_h, 'x_own': xk,
            'w_lT': w_lT_h, 'w_rT': w_rT_h, 'w_eT': w_eT_h,
            'att_exp': att_exp_h, 'bias_lr': bias_lr_h,
        }
        if aff_h is not None:
            im['aff'] = aff_h
        in_maps.append(im)

    res = run_bass_kernel_spmd(nc, in_maps, list(range(N_CORES)))
    outs = []
    for k in range(N_CORES):
        n_own = min(NPC, N - k * NPC)
        outs.append(res.results[k]['out'][:n_own])
    return np.concatenate(outs, axis=0)


# revision 14
# speedup vs baseline: 1.1119x; 1.1119x over previous
"""GATv2 block kernel for 8 Trainium2 NeuronCores (Bass/Tile) — v2.

Strategy (graph/data parallel over destination nodes):
  - Host sorts edges by destination, shards destination nodes across the
    8 cores (6250 nodes each, padded to 6272 = 49 tiles of 128).
  - Per destination-node tile, edges are padded to multiples of 128
    ("chunks"); chunk counts per tile are maxed across cores so one SPMD
    program serves all 8 cores.
  - Host supplies x[src] AND x[dst] pre-gathered + transposed (fp8) so
    every linear transform on device is a matmul with a constant
    stationary operand (no indicator gathers, no partition broadcasts).
  - fp8 stationaries get 4x fast-weight-load; segment softmax + scatter
    are matmuls against an edge->node indicator built by GpSimd.
"""

import numpy as np
import ml_dtypes

BF16 = ml_dtypes.bfloat16
FP8 = ml_dtypes.float8_e4m3

P = 128
HEADS = 4
HEAD_DIM = 32
OUT_DIM = 128
IN_DIM = 128
EDGE_DIM = 10
NEG_SLOPE = 0.2
LN_EPS = 1e-5
N_CORES = 8
SUPER = 4  # chunks per superchunk (free dim 512)

# engine for the indicator build: 'gpsimd' or 'vector'
I_ENGINE = 'vector'

_CACHE = {}

_PATCHED = []


def _enable_ldw_opt():
    # walrus LDWEIGHTS double-buffering: lets weight loads overlap in-flight
    # matmuls instead of serializing every LDW+MM pair.
    if _PATCHED:
        return
    from concourse import bass_utils as bu
    orig = bu.run_command

    def run_command(argv, **kwargs):
        argv = ['--enable-ldw-opt=true' if a == '--enable-ldw-opt=false' else a
                for a in argv]
        return orig(argv, **kwargs)

    bu.run_command = run_command
    _PATCHED.append(True)


def _build_program(C_list, trivial_affine):
    import concourse.bacc as bacc
    import concourse.bass as bass
    import concourse.tile as tile
    from concourse import mybir

    f32 = mybir.dt.float32
    bf16 = mybir.dt.bfloat16
    fp8 = mybir.dt.float8e4
    AT = mybir.ActivationFunctionType
    OP = mybir.AluOpType

    NT = len(C_list)                       # node tiles per core
    CMAX = max(C_list)
    TOTAL_CHUNKS = sum(C_list)
    NPC_PAD = NT * P
    EW = TOTAL_CHUNKS * P                  # padded edges per core

    nc = bacc.Bacc('TRN2', target_bir_lowering=False, debug=False,
                   enable_asserts=True, num_devices=N_CORES)

    # ---- external inputs ----
    x_srcT = nc.dram_tensor('x_srcT', [P, EW], fp8, kind='ExternalInput')
    x_dstT = nc.dram_tensor('x_dstT', [P, EW], fp8, kind='ExternalInput')
    attrT = nc.dram_tensor('attrT', [EDGE_DIM, EW], fp8, kind='ExternalInput')
    ind = nc.dram_tensor('ind', [P, EW], fp8, kind='ExternalInput')
    x_own = nc.dram_tensor('x_own', [NPC_PAD, P], f32, kind='ExternalInput')
    w_lT = nc.dram_tensor('w_lT', [P, P], fp8, kind='ExternalInput')
    w_rT = nc.dram_tensor('w_rT', [P, P], fp8, kind='ExternalInput')
    w_eT = nc.dram_tensor('w_eT', [EDGE_DIM, P], fp8, kind='ExternalInput')
    att_exp = nc.dram_tensor('att_exp', [P, HEADS], fp8, kind='ExternalInput')
    bias_lr = nc.dram_tensor('bias_lr', [P, 1], f32, kind='ExternalInput')
    aff = None
    if not trivial_affine:
        # rows: b_l bcast, conv_bias bcast, gamma bcast, beta bcast
        aff = nc.dram_tensor('aff', [P, 4 * P], f32, kind='ExternalInput')

    out_d = nc.dram_tensor('out', [NPC_PAD, P], f32, kind='ExternalOutput')

    from concourse import library_config
    with tile.TileContext(nc) as tc:
        nc.gpsimd.load_library(library_config.standard)
        with tc.tile_pool(name='const', bufs=1) as cp:
            c_wlT = cp.tile([P, P], fp8)
            nc.sync.dma_start(c_wlT[:], w_lT[:])
            c_wrT = cp.tile([P, P], fp8)
            nc.sync.dma_start(c_wrT[:], w_rT[:])
            c_weT = cp.tile([EDGE_DIM, P], fp8)
            nc.sync.dma_start(c_weT[:], w_eT[:])
            c_att = cp.tile([P, HEADS], fp8)
            nc.sync.dma_start(c_att[:], att_exp[:])
            c_blr = cp.tile([P, 1], f32)
            nc.sync.dma_start(c_blr[:], bias_lr[:])
            c_aff = None
            if aff is not None:
                c_aff = cp.tile([P, 4 * P], f32)
                nc.sync.dma_start(c_aff[:], aff[:])

            with tc.tile_pool(name='persist', bufs=1) as pp:
                ubuf = pp.tile([P, NT * 132], f32)     # unnorm(128)+denom(4)
                hbuf = pp.tile([P, NT * P], f32)       # post-residual h
                stats = pp.tile([P, NT * 2], f32)      # mean, var interleaved

                # ---------- edge pipeline ----------
                with tc.tile_pool(name='eload', bufs=3) as lp, \
                     tc.tile_pool(name='ework', bufs=3) as wp, \
                     tc.tile_pool(name='psA', bufs=2, space='PSUM') as psA, \
                     tc.tile_pool(name='psX', bufs=2, space='PSUM') as psX, \
                     tc.tile_pool(name='psC', bufs=2, space='PSUM') as psC, \
                     tc.tile_pool(name='psO', bufs=2, space='PSUM') as psO:
                    chunk_base = 0
                    for t in range(NT):
                        Ct = C_list[t]
                        te0 = chunk_base * P
                        TW = Ct * P
                        xsT_t = lp.tile([P, CMAX * P], fp8, tag='xsT')
                        nc.sync.dma_start(xsT_t[:, :TW], x_srcT[:, te0:te0 + TW])
                        xdT_t = lp.tile([P, CMAX * P], fp8, tag='xdT')
                        nc.sync.dma_start(xdT_t[:, :TW], x_dstT[:, te0:te0 + TW])
                        atr_t = lp.tile([EDGE_DIM, CMAX * P], fp8, tag='atr')
                        nc.sync.dma_start(atr_t[:, :TW], attrT[:, te0:te0 + TW])
                        # indicator I[e, c, n] = (dst_local[e,c] == n),
                        # prebuilt on host
                        I_t = lp.tile([P, CMAX, P], fp8, tag='I')
                        nc.sync.dma_start(
                            I_t[:, :Ct, :].rearrange('p c n -> p (c n)'),
                            ind[:, te0:te0 + TW])

                        ps_out = psO.tile([P, 132], f32, tag='out')
                        n_super = (Ct + SUPER - 1) // SUPER
                        for s in range(n_super):
                            nch = min(SUPER, Ct - s * SUPER)
                            W = nch * P
                            o0 = s * SUPER * P
                            xsT = xsT_t[:, o0:o0 + W]
                            xdT = xdT_t[:, o0:o0 + W]
                            atr = atr_t[:, o0:o0 + W]

                            # s^T = xl[src]^T + ea^T + xr[dst]^T (feature-major)
                            ps_sT = psA.tile([P, SUPER * P], f32, tag='sT')
                            nc.tensor.matmul(ps_sT[:, :W], lhsT=c_wlT[:],
                                             rhs=xsT, start=True, stop=False)
                            nc.tensor.matmul(ps_sT[:, :W], lhsT=c_weT[:],
                                             rhs=atr, start=False, stop=False)
                            nc.tensor.matmul(ps_sT[:, :W], lhsT=c_wrT[:],
                                             rhs=xdT, start=False, stop=True)

                            # m = lrelu(s + (b_l+b_r))  (bias per feature row)
                            m = wp.tile([P, SUPER * P], fp8, tag='m')
                            nc.scalar.activation(m[:, :W], ps_sT[:, :W], AT.Prelu,
                                                 bias=c_blr[:], alpha=NEG_SLOPE)

                            # logits edge-major: [128e, 4] per chunk
                            ps_ex = psC.tile([P, SUPER * HEADS], f32, tag='lgex')
                            for j in range(nch):
                                nc.tensor.matmul(
                                    ps_ex[:, j * HEADS:(j + 1) * HEADS],
                                    lhsT=m[:, j * P:(j + 1) * P],
                                    rhs=c_att[:], start=True, stop=True)

                            # msg = [xj * ex_bcast | ex]  -> [128, nch, 132]
                            msg = wp.tile([P, SUPER, 132], fp8, tag='msg')
                            # ex = exp(logits), straight into msg cols 128:132
                            nc.scalar.activation(
                                msg[:, :nch, P:P + HEADS],
                                ps_ex[:, :nch * HEADS].rearrange(
                                    'p (c h) -> p c h', c=nch),
                                AT.Exp)

                            # xj edge-major [e, f]
                            ps_xj = psX.tile([P, SUPER * P], f32, tag='xj')
                            for j in range(nch):
                                nc.tensor.matmul(
                                    ps_xj[:, j * P:(j + 1) * P],
                                    lhsT=xsT[:, j * P:(j + 1) * P],
                                    rhs=c_wlT[:], start=True, stop=True)

                            xj_v = ps_xj[:, :W].rearrange(
                                'p (c h d) -> p c h d', c=nch, h=HEADS)
                            if aff is not None:
                                # general b_l: xj += b_l (broadcast over rows)
                                xj_sb = wp.tile([P, SUPER * P], f32, tag='xjb')
                                blv = c_aff[:, 0:P][:, None, :].to_broadcast(
                                    [P, nch, P])
                                nc.vector.tensor_tensor(
                                    out=xj_sb[:, :W].rearrange(
                                        'p (c f) -> p c f', c=nch),
                                    in0=ps_xj[:, :W].rearrange(
                                        'p (c f) -> p c f', c=nch),
                                    in1=blv, op=OP.add)
                                xj_v = xj_sb[:, :W].rearrange(
                                    'p (c h d) -> p c h d', c=nch, h=HEADS)
                            ex_v = (msg[:, :nch, P:P + HEADS]
                                    [:, :, :, None].to_broadcast(
                                        [P, nch, HEADS, HEAD_DIM]))
                            nc.vector.tensor_tensor(
                                out=msg[:, :nch, 0:P].rearrange(
                                    'p c (h d) -> p c h d', h=HEADS),
                                in0=xj_v, in1=ex_v, op=OP.mult)

                            # scatter: ps_out[n, :] += I^T @ msg
                            # adjacent chunks pair into one fp8 DoubleRow
                            # matmul (both contract into the same accumulator)
                            j = 0
                            while j < nch:
                                first = (s == 0 and j == 0)
                                if j + 1 < nch:
                                    last = (s == n_super - 1 and j + 2 >= nch)
                                    nc.tensor.matmul(
                                        ps_out[:],
                                        lhsT=I_t[:, s * SUPER + j:
                                                 s * SUPER + j + 2, :],
                                        rhs=msg[:, j:j + 2, :],
                                        start=first, stop=last,
                                        perf_mode=mybir.MatmulPerfMode.DoubleRow)
                                    j += 2
                                else:
                                    last = (s == n_super - 1)
                                    nc.tensor.matmul(
                                        ps_out[:],
                                        lhsT=I_t[:, s * SUPER + j, :],
                                        rhs=msg[:, j, :],
                                        start=first, stop=last)
                                    j += 1
                        nc.scalar.copy(
                            ubuf[:, t * 132:(t + 1) * 132], ps_out[:])
                        chunk_base += Ct

                # ---------- tail: normalize + silu + residual + LN ----------
                with tc.tile_pool(name='tail', bufs=3) as tp:
                    for t in range(NT):
                        u_sl = ubuf[:, t * 132:t * 132 + P]
                        d_sl = ubuf[:, t * 132 + P:t * 132 + P + HEADS]
                        rv = tp.tile([P, HEADS], f32, tag='rv')
                        nc.vector.tensor_scalar(
                            out=rv[:], in0=d_sl, scalar1=1e-16, scalar2=None,
                            op0=OP.add)
                        rvi = tp.tile([P, HEADS], f32, tag='rvi')
                        nc.vector.reciprocal(rvi[:], rv[:])
                        u = tp.tile([P, P], f32, tag='u')
                        rvi_v = rvi[:, :, None].to_broadcast(
                            [P, HEADS, HEAD_DIM])
                        nc.vector.tensor_tensor(
                            out=u[:].rearrange('p (h d) -> p h d', h=HEADS),
                            in0=u_sl.rearrange('p (h d) -> p h d', h=HEADS),
                            in1=rvi_v, op=OP.mult)
                        if aff is not None:
                            nc.vector.tensor_tensor(
                                out=u[:], in0=u[:], in1=c_aff[:, P:2 * P],
                                op=OP.add)
                        ss = tp.tile([P, P], f32, tag='ss')
                        nc.scalar.activation(ss[:], u[:], AT.Silu)
                        xo = tp.tile([P, P], f32, tag='xo')
                        nc.gpsimd.dma_start(xo[:], x_own[t * P:(t + 1) * P, :])
                        h_sl = hbuf[:, t * P:(t + 1) * P]
                        nc.vector.tensor_tensor(out=h_sl, in0=ss[:], in1=xo[:],
                                                op=OP.add)
                        bs = tp.tile([P, 6], f32, tag='bs')
                        nc.vector.bn_stats(bs[:], h_sl)
                        nc.vector.bn_aggr(stats[:, t * 2:t * 2 + 2], bs[:])

                    veps = tp.tile([P, NT], f32, tag='veps')
                    var_v = stats[:].rearrange('p (t k) -> p t k', k=2)[:, :, 1]
                    nc.vector.tensor_scalar(out=veps[:], in0=var_v,
                                            scalar1=LN_EPS, scalar2=None,
                                            op0=OP.add)
                    vinv = tp.tile([P, NT], f32, tag='vinv')
                    nc.vector.reciprocal(vinv[:], veps[:])
                    rstd = tp.tile([P, NT], f32, tag='rstd')
                    nc.scalar.activation(rstd[:], vinv[:], AT.Sqrt)

                    for t in range(NT):
                        o = tp.tile([P, P], f32, tag='o')
                        nc.vector.tensor_scalar(
                            out=o[:], in0=hbuf[:, t * P:(t + 1) * P],
                            scalar1=stats[:, t * 2:t * 2 + 1],
                            scalar2=rstd[:, t:t + 1],
                            op0=OP.subtract, op1=OP.mult)
                        if aff is not None:
                            nc.vector.tensor_tensor(
                                out=o[:], in0=o[:], in1=c_aff[:, 2 * P:3 * P],
                                op=OP.mult)
                            nc.vector.tensor_tensor(
                                out=o[:], in0=o[:], in1=c_aff[:, 3 * P:4 * P],
                                op=OP.add)
                        nc.gpsimd.dma_start(out_d[t * P:(t + 1) * P, :], o[:])

    nc.compile()
    return nc


def kernel(x, edge_index, edge_attr, w_l, b_l, w_r, b_r, w_e, att,
           conv_bias, ln_gamma, ln_beta):
    from concourse.bass_utils import run_bass_kernel_spmd

    x = np.asarray(x, dtype=np.float32)
    edge_index = np.asarray(edge_index)
    edge_attr = np.asarray(edge_attr, dtype=np.float32)
    w_l = np.asarray(w_l, dtype=np.float32)
    b_l = np.asarray(b_l, dtype=np.float32)
    w_r = np.asarray(w_r, dtype=np.float32)
    b_r = np.asarray(b_r, dtype=np.float32)
    w_e = np.asarray(w_e, dtype=np.float32)
    att = np.asarray(att, dtype=np.float32)
    conv_bias = np.asarray(conv_bias, dtype=np.float32)
    ln_gamma = np.asarray(ln_gamma, dtype=np.float32)
    ln_beta = np.asarray(ln_beta, dtype=np.float32)

    N = x.shape[0]
    E = edge_index.shape[1]
    NPC = (N + N_CORES - 1) // N_CORES          # 6250
    NT = (NPC + P - 1) // P                     # 49
    NPC_PAD = NT * P                            # 6272

    src = edge_index[0].astype(np.int64)
    dst = edge_index[1].astype(np.int64)
    core = np.minimum(dst // NPC, N_CORES - 1)

    trivial_affine = (not b_l.any()) and (not conv_bias.any()) and \
        np.all(ln_gamma == 1.0) and (not ln_beta.any())

    # per (core, tile) edge lists, sorted by dst
    order = np.lexsort((dst,))
    src_s, dst_s, core_s = src[order], dst[order], core[order]
    attr_s = edge_attr[order]
    tile_of = (dst_s - core_s * NPC) // P

    counts = np.zeros((N_CORES, NT), dtype=np.int64)
    np.add.at(counts, (core_s, tile_of), 1)
    C_list = [int(max(1, np.max((counts[:, t] + P - 1) // P)))
              for t in range(NT)]
    TOTAL_CHUNKS = sum(C_list)
    EW = TOTAL_CHUNKS * P

    key = (tuple(C_list), trivial_affine)
    if key in _CACHE:
        nc = _CACHE[key]
    else:
        nc = _build_program(C_list, trivial_affine)
        _CACHE[key] = nc

    # chunk start offsets per tile
    tile_chunk0 = np.zeros(NT, dtype=np.int64)
    acc = 0
    for t in range(NT):
        tile_chunk0[t] = acc
        acc += C_list[t]

    # consts shared by all cores
    w_lT_h = np.ascontiguousarray(w_l.T).astype(FP8)
    w_rT_h = np.ascontiguousarray(w_r.T).astype(FP8)
    w_eT_h = np.ascontiguousarray(w_e.T).astype(FP8)
    att_exp_h = np.zeros((P, HEADS), dtype=FP8)
    for h in range(HEADS):
        att_exp_h[h * HEAD_DIM:(h + 1) * HEAD_DIM, h] = att[h]
    bias_lr_h = (b_l + b_r)[:, None].astype(np.float32).copy()
    aff_h = None
    if not trivial_affine:
        aff_h = np.concatenate([
            np.broadcast_to(b_l, (P, P)),
            np.broadcast_to(conv_bias, (P, P)),
            np.broadcast_to(ln_gamma, (P, P)),
            np.broadcast_to(ln_beta, (P, P))], axis=1).astype(np.float32).copy()

    x_fp8 = x.astype(FP8)

    in_maps = []
    for k in range(N_CORES):
        sel = core_s == k
        ksrc, kdst, ktile = src_s[sel], dst_s[sel], tile_of[sel]
        kattr = attr_s[sel]
        # position of each edge in the padded layout
        # edges already sorted by dst -> grouped by tile, in order
        pos = np.empty(len(ksrc), dtype=np.int64)
        x_srcT_h = np.zeros((P, EW), dtype=FP8)
        x_dstT_h = np.zeros((P, EW), dtype=FP8)
        attrT_h = np.zeros((EDGE_DIM, EW), dtype=FP8)
        ind_h = np.zeros((P, EW), dtype=FP8)
        for t in range(NT):
            tsel = ktile == t
            n_t = int(tsel.sum())
            base = tile_chunk0[t] * P
            pos[tsel] = base + np.arange(n_t)
        x_srcT_h[:, pos] = x_fp8[ksrc].T
        x_dstT_h[:, pos] = x_fp8[kdst].T
        attrT_h[:, pos] = kattr.T.astype(FP8)
        dloc = (kdst - k * NPC - ktile * P).astype(np.int64)
        # indicator: edge at padded slot pos (row pos%128, chunk pos//128)
        # points at local node dloc -> column (pos//128)*128 + dloc
        ind_h[pos % P, (pos // P) * P + dloc] = 1.0

        xk = np.zeros((NPC_PAD, P), dtype=np.float32)
        n_own = min(NPC, N - k * NPC)
        xk[:n_own] = x[k * NPC:k * NPC + n_own]
        im = {
            'x_srcT': x_srcT_h, 'x_dstT': x_dstT_h, 'attrT': attrT_h,
            'ind': ind_h, 'x_own': xk,
            'w_lT': w_lT_h, 'w_rT': w_rT_h, 'w_eT': w_eT_h,
            'att_exp': att_exp_h, 'bias_lr': bias_lr_h,
        }
        if aff_h is not None:
            im['aff'] = aff_h
        in_maps.append(im)

    res = run_bass_kernel_spmd(nc, in_maps, list(range(N_CORES)))
    outs = []
    for k in range(N_CORES):
        n_own = min(NPC, N - k * NPC)
        outs.append(res.results[k]['out'][:n_own])
    return np.concatenate(outs, axis=0)
